# revision 5
# baseline (speedup 1.0000x reference)
"""Trainium2 Bass kernel: attention layer (B=4, S=2048, D=1024), 8 NeuronCores.

Sharding: data-parallel over (batch, query-half) -> 8 shards. Each core
computes one batch's half of the queries against that batch's full key/value.

Per-core dataflow (all transpose-free; host pre-transposes inputs):
  QT[e,q]   = Wq projection of queries   (fp32r matmuls, full PE rate)
  KT[e,k]   = Wk projection of keys      (fp32r, resident in SBUF)
  ST[k,q]   = scores, k on partitions    (fp32r; streamed to DRAM scratch)
  softmax along k (partition dim): DVE max-tree across k-tiles +
    gpsimd C-axis reduce -> row max m[1,q]; broadcast via K=1 ones-matmul;
    E = exp(ST - m) stored bf16
  V[k,e]    = value projection           (bf16)
  l[q]      = sum_k E  via N=1 ones-matmuls (fp32 PSUM accumulate)
  O[q,e]    = (E.T @ V) * (1/l)          (bf16 matmuls, fp32 drain)

float32r matmul measured on silicon: ~1 cycle/row (vs fp32's 4) with
~1.5e-4 relative precision -- enough for the unscaled (near-one-hot)
softmax here (simulated end-to-end rel err ~8e-4 vs fp32 reference).
QT and ST spill to DRAM scratch to keep peak SBUF under 192KB/partition.
"""

import numpy as np
import ml_dtypes
from contextlib import ExitStack

import concourse.bass as bass
import concourse.tile as tile
from concourse import bacc, mybir
from concourse.bass import ts
from concourse.bass_utils import run_bass_kernel_spmd

B, S, D = 4, 2048, 1024
N_CORES = 8
SQ = S // 2            # 1024 query rows per core
P = 128                # partitions
NE = D // P            # 8 e-tiles
ND = D // P            # 8 d-tiles
NK = S // P            # 16 k-tiles
NQC = SQ // P          # 8 q-chunks
F32R = mybir.dt.float32r
F32 = mybir.dt.float32
BF16 = mybir.dt.bfloat16

_NC_CACHE = {}


def _build():
    if "nc" in _NC_CACHE:
        return _NC_CACHE["nc"]
    nc = bacc.Bacc("TRN2", target_bir_lowering=False, debug=False,
                   num_devices=N_CORES)

    qT = nc.dram_tensor("qT", [D, SQ], F32R, kind="ExternalInput")
    kT = nc.dram_tensor("kT", [D, S], F32R, kind="ExternalInput")
    wqT = nc.dram_tensor("wqT", [D, D], F32R, kind="ExternalInput")
    wkT = nc.dram_tensor("wkT", [D, D], F32R, kind="ExternalInput")
    kTb = nc.dram_tensor("kTb", [D, S], BF16, kind="ExternalInput")
    wvTb = nc.dram_tensor("wvTb", [D, D], BF16, kind="ExternalInput")
    out = nc.dram_tensor("out", [SQ, D], F32, kind="ExternalOutput")

    with tile.TileContext(nc) as tc:
        with ExitStack() as ctx:
            psum = ctx.enter_context(tc.tile_pool(name="psum", bufs=4, space="PSUM"))
            psl = ctx.enter_context(tc.tile_pool(name="psl", bufs=2, space="PSUM"))
            dram = ctx.enter_context(tc.tile_pool(name="dram", bufs=1, space="DRAM"))
            consts = ctx.enter_context(tc.tile_pool(name="consts", bufs=1))
            outp = ctx.enter_context(tc.tile_pool(name="outp", bufs=3))

            ones_f = consts.tile([1, P], F32)
            nc.gpsimd.memset(ones_f[:], 1.0)
            ones_r = consts.tile([1, P], F32R)
            nc.vector.tensor_copy(ones_r[:], ones_f[:])
            ones_b = consts.tile([P, 1], BF16)
            nc.gpsimd.memset(ones_b[:], 1.0)

            qt_spill = [dram.tile([P, SQ], F32R, tag="qtsp", name=f"qtsp{i}")
                        for i in range(NE)]
            st_spill = [dram.tile([P, SQ], F32, tag="stsp", name=f"stsp{i}")
                        for i in range(NK)]

            # ---- P1: QT[e,q] projection; spill to DRAM ----------------
            with tc.tile_pool(name="qin", bufs=ND) as qin, \
                 tc.tile_pool(name="wq", bufs=ND) as wq, \
                 tc.tile_pool(name="qtsb", bufs=3) as qtsb:
                qts = [qin.tile([P, SQ], F32R, tag="qin", name=f"qin{i}")
                       for i in range(ND)]
                wqs = [wq.tile([P, D], F32R, tag="wq", name=f"wq{i}")
                       for i in range(ND)]
                for d in range(ND):
                    nc.sync.dma_start(qts[d][:], qT.ap()[ts(d, P), :])
                    nc.sync.dma_start(wqs[d][:], wqT.ap()[ts(d, P), :])
                for e in range(NE):
                    qt_e = qtsb.tile([P, SQ], F32R, tag="qtsb", name=f"qtsb{e}")
                    for qh in range(SQ // 512):
                        ps = psum.tile([P, 512], F32, tag="mm", name=f"ps_q{e}_{qh}")
                        for d in range(ND):
                            nc.tensor.matmul(ps[:], wqs[d][:, ts(e, P)],
                                             qts[d][:, ts(qh, 512)],
                                             start=(d == 0), stop=(d == ND - 1))
                        nc.vector.tensor_copy(qt_e[:, ts(qh, 512)], ps[:])
                    nc.sync.dma_start(qt_spill[e][:], qt_e[:])

            maxp = ctx.enter_context(tc.tile_pool(name="maxp", bufs=1))
            macc = maxp.tile([P, SQ], F32)

            # ---- P2: KT[e,k] projection; resident. P3 nested ----------
            with tc.tile_pool(name="ktp", bufs=NE) as ktp:
                KT = [ktp.tile([P, S], F32R, tag="kt", name=f"kt{i}")
                      for i in range(NE)]
                with tc.tile_pool(name="kin", bufs=ND) as kin, \
                     tc.tile_pool(name="wk", bufs=ND) as wk:
                    kts = [kin.tile([P, S], F32R, tag="kin", name=f"kin{i}")
                           for i in range(ND)]
                    wks = [wk.tile([P, D], F32R, tag="wk", name=f"wk{i}")
                           for i in range(ND)]
                    for d in range(ND):
                        nc.sync.dma_start(kts[d][:], kT.ap()[ts(d, P), :])
                        nc.sync.dma_start(wks[d][:], wkT.ap()[ts(d, P), :])
                    for e in range(NE):
                        for kc in range(S // 512):
                            ps = psum.tile([P, 512], F32, tag="mm",
                                           name=f"ps_k{e}_{kc}")
                            for d in range(ND):
                                nc.tensor.matmul(ps[:], wks[d][:, ts(e, P)],
                                                 kts[d][:, ts(kc, 512)],
                                                 start=(d == 0), stop=(d == ND - 1))
                            nc.vector.tensor_copy(KT[e][:, ts(kc, 512)], ps[:])

                # ---- P3: scores ST[k,q]; max tree; spill ST -----------
                with tc.tile_pool(name="qtin", bufs=NE) as qtin, \
                     tc.tile_pool(name="stb", bufs=3) as stb:
                    qte = [qtin.tile([P, SQ], F32R, tag="qtin", name=f"qtin{i}")
                           for i in range(NE)]
                    for e in range(NE):
                        nc.sync.dma_start(qte[e][:], qt_spill[e][:])
                    for k in range(NK):
                        st_k = stb.tile([P, SQ], F32, tag="stb", name=f"stb{k}")
                        for qh in range(SQ // 512):
                            ps = psum.tile([P, 512], F32, tag="mm",
                                           name=f"ps_s{k}_{qh}")
                            for e in range(NE):
                                nc.tensor.matmul(ps[:], KT[e][:, ts(k, P)],
                                                 qte[e][:, ts(qh, 512)],
                                                 start=(e == 0), stop=(e == NE - 1))
                            nc.vector.tensor_copy(st_k[:, ts(qh, 512)], ps[:])
                        if k == 0:
                            nc.vector.tensor_copy(macc[:], st_k[:])
                        else:
                            nc.vector.tensor_max(macc[:], macc[:], st_k[:])
                        nc.sync.dma_start(st_spill[k][:], st_k[:])

            # ---- P3b: partition-dim max -> broadcast row max ----------
            m_row = maxp.tile([1, SQ], F32)
            nc.gpsimd.tensor_reduce(m_row[:], macc[:], mybir.AxisListType.C,
                                    op=mybir.AluOpType.max)
            m_row_r = maxp.tile([1, SQ], F32R)
            nc.vector.tensor_copy(m_row_r[:], m_row[:])
            m_bc = maxp.tile([P, SQ], F32)
            for qh in range(SQ // 512):
                ps = psum.tile([P, 512], F32, tag="mm", name=f"ps_m{qh}")
                nc.tensor.matmul(ps[:], ones_r[:], m_row_r[:, ts(qh, 512)],
                                 start=True, stop=True)
                nc.vector.tensor_copy(m_bc[:, ts(qh, 512)], ps[:])

            # ---- P4a: E = exp(ST - m) in bf16 -------------------------
            ep = ctx.enter_context(tc.tile_pool(name="ep", bufs=NK))
            E = [ep.tile([P, SQ], BF16, tag="e", name=f"e{i}") for i in range(NK)]
            with tc.tile_pool(name="stin", bufs=3) as stin, \
                 tc.tile_pool(name="subp", bufs=3) as subp:
                for k in range(NK):
                    st_k = stin.tile([P, SQ], F32, tag="stin", name=f"stin{k}")
                    nc.sync.dma_start(st_k[:], st_spill[k][:])
                    sub = subp.tile([P, SQ], F32, tag="sub", name=f"sub{k}")
                    nc.vector.tensor_sub(sub[:], st_k[:], m_bc[:])
                    nc.scalar.activation(E[k][:], sub[:],
                                         mybir.ActivationFunctionType.Exp)

            # ---- P4b: V[k,e] projection (bf16) ------------------------
            vp = ctx.enter_context(tc.tile_pool(name="vp", bufs=NK))
            V = [vp.tile([P, D], BF16, tag="v", name=f"v{i}") for i in range(NK)]
            with tc.tile_pool(name="kbin", bufs=ND) as kbin, \
                 tc.tile_pool(name="wvb", bufs=ND) as wvb:
                kbs = [kbin.tile([P, S], BF16, tag="kbin", name=f"kbin{i}")
                       for i in range(ND)]
                wvs = [wvb.tile([P, D], BF16, tag="wvb", name=f"wvb{i}")
                       for i in range(ND)]
                for d in range(ND):
                    nc.sync.dma_start(kbs[d][:], kTb.ap()[ts(d, P), :])
                    nc.sync.dma_start(wvs[d][:], wvTb.ap()[ts(d, P), :])
                for k in range(NK):
                    for eh in range(D // 512):
                        ps = psum.tile([P, 512], F32, tag="mm",
                                       name=f"ps_v{k}_{eh}")
                        for d in range(ND):
                            nc.tensor.matmul(ps[:], kbs[d][:, ts(k, P)],
                                             wvs[d][:, ts(eh, 512)],
                                             start=(d == 0), stop=(d == ND - 1))
                        nc.vector.tensor_copy(V[k][:, ts(eh, 512)], ps[:])

            # ---- P4c: l[q] row sums + reciprocal ----------------------
            lp = ctx.enter_context(tc.tile_pool(name="lp", bufs=NQC))
            recip_l = [lp.tile([P, 1], F32, tag="recip", name=f"recip{i}")
                       for i in range(NQC)]
            for qc in range(NQC):
                pl = psl.tile([P, 1], F32, tag="lsum", name=f"pl{qc}")
                for k in range(NK):
                    nc.tensor.matmul(pl[:], E[k][:, ts(qc, P)], ones_b[:],
                                     start=(k == 0), stop=(k == NK - 1))
                nc.vector.reciprocal(recip_l[qc][:], pl[:])

            # ---- P4d: O = (E.T @ V) * recip_l -> DRAM -----------------
            for qc in range(NQC):
                for eh in range(D // 512):
                    ps = psum.tile([P, 512], F32, tag="mm", name=f"ps_o{qc}_{eh}")
                    for k in range(NK):
                        nc.tensor.matmul(ps[:], E[k][:, ts(qc, P)],
                                         V[k][:, ts(eh, 512)],
                                         start=(k == 0), stop=(k == NK - 1))
                    ot = outp.tile([P, 512], F32, tag="ot", name=f"ot{qc}_{eh}")
                    nc.vector.tensor_scalar_mul(ot[:], ps[:], recip_l[qc][:])
                    nc.sync.dma_start(out.ap()[ts(qc, P), ts(eh, 512)], ot[:])

    nc.compile()
    _NC_CACHE["nc"] = nc
    return nc


def kernel(query, key, Wq, Wk, Wv):
    query = np.asarray(query, dtype=np.float32)
    key = np.asarray(key, dtype=np.float32)
    wqT = np.ascontiguousarray(np.asarray(Wq, dtype=np.float32).T)
    wkT = np.ascontiguousarray(np.asarray(Wk, dtype=np.float32).T)
    wvTb = np.ascontiguousarray(np.asarray(Wv, dtype=np.float32).T).astype(
        ml_dtypes.bfloat16)

    in_maps = []
    for c in range(N_CORES):
        b, h = c // 2, c % 2
        qTn = np.ascontiguousarray(query[b, h * SQ:(h + 1) * SQ, :].T)
        kTn = np.ascontiguousarray(key[b].T)
        in_maps.append({
            "qT": qTn, "kT": kTn, "wqT": wqT, "wkT": wkT,
            "kTb": kTn.astype(ml_dtypes.bfloat16), "wvTb": wvTb,
        })

    nc = _build()
    res = run_bass_kernel_spmd(nc, in_maps, core_ids=list(range(N_CORES)))
    outv = np.empty((B, S, D), dtype=np.float32)
    for c in range(N_CORES):
        b, h = c // 2, c % 2
        outv[b, h * SQ:(h + 1) * SQ, :] = res.results[c]["out"]
    return outv


# revision 7
# speedup vs baseline: 1.0600x; 1.0600x over previous
"""Trainium2 Bass kernel: attention layer (B=4, S=2048, D=1024), 8 NeuronCores.

Sharding: data-parallel over (batch, query-half) -> 8 shards. Each core
computes one batch's half of the queries against that batch's full key/value.

Per-core dataflow (all transpose-free; host pre-transposes inputs):
  QT[e,q]   = Wq projection of queries   (fp32r matmuls, full PE rate)
  KT[e,k]   = Wk projection of keys      (fp32r; spilled to DRAM, streamed
              back per k-tile as [e,128] slices during the score phase)
  ST[k,q]   = scores, k on partitions    (fp32r; streamed to DRAM scratch)
  softmax along k (partition dim): DVE max-tree across k-tiles + 7-step
    DMA-shift partition halving -> row max m[1,q]; broadcast via K=1
    ones-matmul; E = exp(ST - m) stored bf16
  V[k,e]    = value projection (bf16), emitted between scores and softmax
              so the PE stays busy during the max-reduce latency
  l[q]      = sum_k E  via N=1 ones-matmuls (fp32 PSUM accumulate)
  O[q,e]    = (E.T @ V) * (1/l)          (bf16 matmuls, fp32 drain)

float32r matmul measured on silicon: ~1 cycle/row (vs fp32's 4) with
~1.5e-4 relative precision -- enough for the unscaled (near-one-hot)
softmax here (simulated end-to-end rel err ~8e-4 vs fp32 reference).
QT/KT/ST spill to DRAM scratch to keep peak SBUF under 192KB/partition.
"""

import numpy as np
import ml_dtypes
from contextlib import ExitStack

import concourse.bass as bass
import concourse.tile as tile
from concourse import bacc, mybir
from concourse.bass import ts
from concourse.bass_utils import run_bass_kernel_spmd

B, S, D = 4, 2048, 1024
N_CORES = 8
SQ = S // 2            # 1024 query rows per core
P = 128                # partitions
NE = D // P            # 8 e-tiles
ND = D // P            # 8 d-tiles
NK = S // P            # 16 k-tiles
NQC = SQ // P          # 8 q-chunks
F32R = mybir.dt.float32r
F32 = mybir.dt.float32
BF16 = mybir.dt.bfloat16

_NC_CACHE = {}


def _build():
    if "nc" in _NC_CACHE:
        return _NC_CACHE["nc"]
    nc = bacc.Bacc("TRN2", target_bir_lowering=False, debug=False,
                   num_devices=N_CORES)

    qT = nc.dram_tensor("qT", [D, SQ], F32R, kind="ExternalInput")
    kT = nc.dram_tensor("kT", [D, S], F32R, kind="ExternalInput")
    wqT = nc.dram_tensor("wqT", [D, D], F32R, kind="ExternalInput")
    wkT = nc.dram_tensor("wkT", [D, D], F32R, kind="ExternalInput")
    kTb = nc.dram_tensor("kTb", [D, S], BF16, kind="ExternalInput")
    wvTb = nc.dram_tensor("wvTb", [D, D], BF16, kind="ExternalInput")
    out = nc.dram_tensor("out", [SQ, D], F32, kind="ExternalOutput")

    with tile.TileContext(nc) as tc:
        with ExitStack() as ctx:
            psum = ctx.enter_context(tc.tile_pool(name="psum", bufs=4, space="PSUM"))
            psl = ctx.enter_context(tc.tile_pool(name="psl", bufs=2, space="PSUM"))
            dram = ctx.enter_context(tc.tile_pool(name="dram", bufs=1, space="DRAM"))
            consts = ctx.enter_context(tc.tile_pool(name="consts", bufs=1))
            outp = ctx.enter_context(tc.tile_pool(name="outp", bufs=3))
            maxp = ctx.enter_context(tc.tile_pool(name="maxp", bufs=1))

            ones_f = consts.tile([1, P], F32)
            nc.gpsimd.memset(ones_f[:], 1.0)
            ones_r = consts.tile([1, P], F32R)
            nc.vector.tensor_copy(ones_r[:], ones_f[:])
            ones_b = consts.tile([P, 1], BF16)
            nc.gpsimd.memset(ones_b[:], 1.0)

            macc = maxp.tile([P, SQ], F32)

            qt_spill = [dram.tile([P, SQ], F32R, tag="qtsp", name=f"qtsp{i}")
                        for i in range(NE)]
            kt_spill = [dram.tile([P, S], F32R, tag="ktsp", name=f"ktsp{i}")
                        for i in range(NE)]
            st_spill = [dram.tile([P, SQ], F32, tag="stsp", name=f"stsp{i}")
                        for i in range(NK)]

            # ---- P1: QT[e,q] projection; spill to DRAM ----------------
            with tc.tile_pool(name="qin", bufs=ND) as qin, \
                 tc.tile_pool(name="wq", bufs=ND) as wq, \
                 tc.tile_pool(name="qtsb", bufs=3) as qtsb:
                qts = [qin.tile([P, SQ], F32R, tag="qin", name=f"qin{i}")
                       for i in range(ND)]
                wqs = [wq.tile([P, D], F32R, tag="wq", name=f"wq{i}")
                       for i in range(ND)]
                for d in range(ND):
                    nc.sync.dma_start(qts[d][:], qT.ap()[ts(d, P), :])
                    nc.sync.dma_start(wqs[d][:], wqT.ap()[ts(d, P), :])
                for e in range(NE):
                    qt_e = qtsb.tile([P, SQ], F32R, tag="qtsb", name=f"qtsb{e}")
                    for qh in range(SQ // 512):
                        ps = psum.tile([P, 512], F32, tag="mm", name=f"ps_q{e}_{qh}")
                        for d in range(ND):
                            nc.tensor.matmul(ps[:], wqs[d][:, ts(e, P)],
                                             qts[d][:, ts(qh, 512)],
                                             start=(d == 0), stop=(d == ND - 1))
                        nc.vector.tensor_copy(qt_e[:, ts(qh, 512)], ps[:])
                    nc.sync.dma_start(qt_spill[e][:], qt_e[:])

            # ---- P2: KT[e,k] projection; spill to DRAM ----------------
            with tc.tile_pool(name="kin", bufs=ND) as kin, \
                 tc.tile_pool(name="wk", bufs=ND) as wk, \
                 tc.tile_pool(name="ktb", bufs=3) as ktb:
                kts = [kin.tile([P, S], F32R, tag="kin", name=f"kin{i}")
                       for i in range(ND)]
                wks = [wk.tile([P, D], F32R, tag="wk", name=f"wk{i}")
                       for i in range(ND)]
                for d in range(ND):
                    nc.sync.dma_start(kts[d][:], kT.ap()[ts(d, P), :])
                    nc.sync.dma_start(wks[d][:], wkT.ap()[ts(d, P), :])
                for e in range(NE):
                    kt_e = ktb.tile([P, S], F32R, tag="ktb", name=f"ktb{e}")
                    for kc in range(S // 512):
                        ps = psum.tile([P, 512], F32, tag="mm",
                                       name=f"ps_k{e}_{kc}")
                        for d in range(ND):
                            nc.tensor.matmul(ps[:], wks[d][:, ts(e, P)],
                                             kts[d][:, ts(kc, 512)],
                                             start=(d == 0), stop=(d == ND - 1))
                        nc.vector.tensor_copy(kt_e[:, ts(kc, 512)], ps[:])
                    nc.sync.dma_start(kt_spill[e][:], kt_e[:])

            # ---- P3: scores ST[k,q] (KT streamed); V proj after -------
            vp = ctx.enter_context(tc.tile_pool(name="vp", bufs=NK))
            V = [vp.tile([P, D], BF16, tag="v", name=f"v{i}") for i in range(NK)]
            with tc.tile_pool(name="qtin", bufs=NE) as qtin, \
                 tc.tile_pool(name="ktsl", bufs=3) as ktsl, \
                 tc.tile_pool(name="stb", bufs=3) as stb, \
                 tc.tile_pool(name="kbin", bufs=ND) as kbin, \
                 tc.tile_pool(name="wvb", bufs=ND) as wvb:
                # prefetch V-projection inputs on the gpsimd DMA queue
                kbs = [kbin.tile([P, S], BF16, tag="kbin", name=f"kbin{i}")
                       for i in range(ND)]
                wvs = [wvb.tile([P, D], BF16, tag="wvb", name=f"wvb{i}")
                       for i in range(ND)]
                for d in range(ND):
                    nc.gpsimd.dma_start(kbs[d][:], kTb.ap()[ts(d, P), :])
                    nc.gpsimd.dma_start(wvs[d][:], wvTb.ap()[ts(d, P), :])

                qte = [qtin.tile([P, SQ], F32R, tag="qtin", name=f"qtin{i}")
                       for i in range(NE)]
                for e in range(NE):
                    nc.sync.dma_start(qte[e][:], qt_spill[e][:])
                for k in range(NK):
                    kt_k = ktsl.tile([P, D], F32R, tag="ktsl", name=f"ktsl{k}")
                    for e in range(NE):
                        nc.sync.dma_start(kt_k[:, ts(e, P)],
                                          kt_spill[e][:, ts(k, P)])
                    st_k = stb.tile([P, SQ], F32, tag="stb", name=f"stb{k}")
                    for qh in range(SQ // 512):
                        ps = psum.tile([P, 512], F32, tag="mm",
                                       name=f"ps_s{k}_{qh}")
                        for e in range(NE):
                            nc.tensor.matmul(ps[:], kt_k[:, ts(e, P)],
                                             qte[e][:, ts(qh, 512)],
                                             start=(e == 0), stop=(e == NE - 1))
                        nc.vector.tensor_copy(st_k[:, ts(qh, 512)], ps[:])
                    if k == 0:
                        nc.vector.tensor_copy(macc[:], st_k[:])
                    else:
                        nc.vector.tensor_max(macc[:], macc[:], st_k[:])
                    nc.sync.dma_start(st_spill[k][:], st_k[:])

                # V projection: no softmax deps -> fills PE during reduce
                for k in range(NK):
                    for eh in range(D // 512):
                        ps = psum.tile([P, 512], F32, tag="mm",
                                       name=f"ps_v{k}_{eh}")
                        for d in range(ND):
                            nc.tensor.matmul(ps[:], kbs[d][:, ts(k, P)],
                                             wvs[d][:, ts(eh, 512)],
                                             start=(d == 0), stop=(d == ND - 1))
                        nc.vector.tensor_copy(V[k][:, ts(eh, 512)], ps[:])

            # ---- P3b: partition halving max -> broadcast row max ------
            tmp = maxp.tile([64, SQ], F32)
            w = 64
            while w >= 1:
                nc.sync.dma_start(tmp[0:w, :], macc[w:2 * w, :])
                nc.vector.tensor_max(macc[0:w, :], macc[0:w, :], tmp[0:w, :])
                w //= 2
            m_row_r = maxp.tile([1, SQ], F32R)
            nc.vector.tensor_copy(m_row_r[:], macc[0:1, :])
            m_bc = maxp.tile([P, SQ], F32)
            for qh in range(SQ // 512):
                ps = psum.tile([P, 512], F32, tag="mm", name=f"ps_m{qh}")
                nc.tensor.matmul(ps[:], ones_r[:], m_row_r[:, ts(qh, 512)],
                                 start=True, stop=True)
                nc.vector.tensor_copy(m_bc[:, ts(qh, 512)], ps[:])

            # ---- P4a: E = exp(ST - m) in bf16 -------------------------
            ep = ctx.enter_context(tc.tile_pool(name="ep", bufs=NK))
            E = [ep.tile([P, SQ], BF16, tag="e", name=f"e{i}") for i in range(NK)]
            with tc.tile_pool(name="stin", bufs=3) as stin, \
                 tc.tile_pool(name="subp", bufs=3) as subp:
                for k in range(NK):
                    st_k = stin.tile([P, SQ], F32, tag="stin", name=f"stin{k}")
                    nc.sync.dma_start(st_k[:], st_spill[k][:])
                    sub = subp.tile([P, SQ], F32, tag="sub", name=f"sub{k}")
                    nc.vector.tensor_sub(sub[:], st_k[:], m_bc[:])
                    nc.scalar.activation(E[k][:], sub[:],
                                         mybir.ActivationFunctionType.Exp)

            # ---- P4c: l[q] row sums + reciprocal ----------------------
            lp = ctx.enter_context(tc.tile_pool(name="lp", bufs=NQC))
            recip_l = [lp.tile([P, 1], F32, tag="recip", name=f"recip{i}")
                       for i in range(NQC)]
            for qc in range(NQC):
                pl = psl.tile([P, 1], F32, tag="lsum", name=f"pl{qc}")
                for k in range(NK):
                    nc.tensor.matmul(pl[:], E[k][:, ts(qc, P)], ones_b[:],
                                     start=(k == 0), stop=(k == NK - 1))
                nc.vector.reciprocal(recip_l[qc][:], pl[:])

            # ---- P4d: O = (E.T @ V) * recip_l -> DRAM -----------------
            for qc in range(NQC):
                for eh in range(D // 512):
                    ps = psum.tile([P, 512], F32, tag="mm", name=f"ps_o{qc}_{eh}")
                    for k in range(NK):
                        nc.tensor.matmul(ps[:], E[k][:, ts(qc, P)],
                                         V[k][:, ts(eh, 512)],
                                         start=(k == 0), stop=(k == NK - 1))
                    ot = outp.tile([P, 512], F32, tag="ot", name=f"ot{qc}_{eh}")
                    nc.vector.tensor_scalar_mul(ot[:], ps[:], recip_l[qc][:])
                    nc.sync.dma_start(out.ap()[ts(qc, P), ts(eh, 512)], ot[:])

    nc.compile()
    _NC_CACHE["nc"] = nc
    return nc


def kernel(query, key, Wq, Wk, Wv):
    query = np.asarray(query, dtype=np.float32)
    key = np.asarray(key, dtype=np.float32)
    wqT = np.ascontiguousarray(np.asarray(Wq, dtype=np.float32).T)
    wkT = np.ascontiguousarray(np.asarray(Wk, dtype=np.float32).T)
    wvTb = np.ascontiguousarray(np.asarray(Wv, dtype=np.float32).T).astype(
        ml_dtypes.bfloat16)

    in_maps = []
    for c in range(N_CORES):
        b, h = c // 2, c % 2
        qTn = np.ascontiguousarray(query[b, h * SQ:(h + 1) * SQ, :].T)
        kTn = np.ascontiguousarray(key[b].T)
        in_maps.append({
            "qT": qTn, "kT": kTn, "wqT": wqT, "wkT": wkT,
            "kTb": kTn.astype(ml_dtypes.bfloat16), "wvTb": wvTb,
        })

    nc = _build()
    res = run_bass_kernel_spmd(nc, in_maps, core_ids=list(range(N_CORES)))
    outv = np.empty((B, S, D), dtype=np.float32)
    for c in range(N_CORES):
        b, h = c // 2, c % 2
        outv[b, h * SQ:(h + 1) * SQ, :] = res.results[c]["out"]
    return outv


# revision 8
# speedup vs baseline: 1.1306x; 1.0666x over previous
"""Trainium2 Bass kernel: attention layer (B=4, S=2048, D=1024), 8 NeuronCores.

Sharding: data-parallel over (batch, query-half) -> 8 shards. Each core
computes one batch's half of the queries against that batch's full key/value.

Per-core dataflow (all transpose-free; host pre-transposes inputs):
  QT[e,q]   = Wq projection of queries   (fp32r matmuls, full PE rate)
  KT[e,k]   = Wk projection of keys      (fp32r; spilled to DRAM, streamed
              back per k-tile as [e,128] slices during the score phase)
  ST[k,q]   = scores, k on partitions    (fp32r; streamed to DRAM scratch)
  softmax along k (partition dim): DVE max-tree across k-tiles + 7-step
    DMA-shift partition halving -> row max m[1,q]; broadcast via K=1
    ones-matmul; E = exp(ST - m) stored bf16
  V[k,e]    = value projection (bf16), emitted between scores and softmax
              so the PE stays busy during the max-reduce latency
  l[q]      = sum_k E  via N=1 ones-matmuls (fp32 PSUM accumulate)
  O[q,e]    = (E.T @ V) * (1/l)          (bf16 matmuls, fp32 drain)

float32r matmul measured on silicon: ~1 cycle/row (vs fp32's 4) with
~1.5e-4 relative precision -- enough for the unscaled (near-one-hot)
softmax here (simulated end-to-end rel err ~8e-4 vs fp32 reference).
QT/KT/ST spill to DRAM scratch to keep peak SBUF under 192KB/partition.
"""

import numpy as np
import ml_dtypes
from contextlib import ExitStack

import concourse.bass as bass
import concourse.tile as tile
from concourse import bacc, mybir
from concourse.bass import ts
from concourse.bass_utils import run_bass_kernel_spmd

B, S, D = 4, 2048, 1024
N_CORES = 8
SQ = S // 2            # 1024 query rows per core
P = 128                # partitions
NE = D // P            # 8 e-tiles
ND = D // P            # 8 d-tiles
NK = S // P            # 16 k-tiles
NQC = SQ // P          # 8 q-chunks
F32R = mybir.dt.float32r
F32 = mybir.dt.float32
BF16 = mybir.dt.bfloat16

_NC_CACHE = {}


def _build():
    if "nc" in _NC_CACHE:
        return _NC_CACHE["nc"]
    nc = bacc.Bacc("TRN2", target_bir_lowering=False, debug=False,
                   num_devices=N_CORES)

    qT = nc.dram_tensor("qT", [D, SQ], F32R, kind="ExternalInput")
    kT = nc.dram_tensor("kT", [D, S], F32R, kind="ExternalInput")
    wqT = nc.dram_tensor("wqT", [D, D], F32R, kind="ExternalInput")
    wkT = nc.dram_tensor("wkT", [D, D], F32R, kind="ExternalInput")
    kTb = nc.dram_tensor("kTb", [D, S], BF16, kind="ExternalInput")
    wvTb = nc.dram_tensor("wvTb", [D, D], BF16, kind="ExternalInput")
    out = nc.dram_tensor("out", [SQ, D], F32, kind="ExternalOutput")

    with tile.TileContext(nc) as tc:
        with ExitStack() as ctx:
            psum = ctx.enter_context(tc.tile_pool(name="psum", bufs=4, space="PSUM"))
            psl = ctx.enter_context(tc.tile_pool(name="psl", bufs=2, space="PSUM"))
            dram = ctx.enter_context(tc.tile_pool(name="dram", bufs=1, space="DRAM"))
            consts = ctx.enter_context(tc.tile_pool(name="consts", bufs=1))
            outp = ctx.enter_context(tc.tile_pool(name="outp", bufs=3))
            maxp = ctx.enter_context(tc.tile_pool(name="maxp", bufs=1))
            qtp = ctx.enter_context(tc.tile_pool(name="qtp", bufs=NE))

            ones_f = consts.tile([1, P], F32)
            nc.gpsimd.memset(ones_f[:], 1.0)
            ones_r = consts.tile([1, P], F32R)
            nc.vector.tensor_copy(ones_r[:], ones_f[:])
            ones_b = consts.tile([P, 1], BF16)
            nc.gpsimd.memset(ones_b[:], 1.0)

            macc = maxp.tile([P, SQ], F32)

            kt_spill = [dram.tile([P, S], F32R, tag="ktsp", name=f"ktsp{i}")
                        for i in range(NE)]
            st_spill = [dram.tile([P, SQ], F32, tag="stsp", name=f"stsp{i}")
                        for i in range(NK)]

            # ---- P1: QT[e,q] projection; stays resident in SBUF -------
            QTr = [qtp.tile([P, SQ], F32R, tag="qtr", name=f"qtr{i}")
                   for i in range(NE)]
            with tc.tile_pool(name="qin", bufs=ND) as qin, \
                 tc.tile_pool(name="wq", bufs=ND) as wq:
                qts = [qin.tile([P, SQ], F32R, tag="qin", name=f"qin{i}")
                       for i in range(ND)]
                wqs = [wq.tile([P, D], F32R, tag="wq", name=f"wq{i}")
                       for i in range(ND)]
                for d in range(ND):
                    nc.sync.dma_start(qts[d][:], qT.ap()[ts(d, P), :])
                    nc.gpsimd.dma_start(wqs[d][:], wqT.ap()[ts(d, P), :])
                for e in range(NE):
                    for qh in range(SQ // 512):
                        ps = psum.tile([P, 512], F32, tag="mm", name=f"ps_q{e}_{qh}")
                        for d in range(ND):
                            nc.tensor.matmul(ps[:], wqs[d][:, ts(e, P)],
                                             qts[d][:, ts(qh, 512)],
                                             start=(d == 0), stop=(d == ND - 1))
                        nc.vector.tensor_copy(QTr[e][:, ts(qh, 512)], ps[:])

            # ---- P2: KT[e,k] projection; spill to DRAM ----------------
            with tc.tile_pool(name="kin", bufs=ND) as kin, \
                 tc.tile_pool(name="wk", bufs=ND) as wk, \
                 tc.tile_pool(name="ktb", bufs=4) as ktb:
                kts = [kin.tile([P, S], F32R, tag="kin", name=f"kin{i}")
                       for i in range(ND)]
                wks = [wk.tile([P, D], F32R, tag="wk", name=f"wk{i}")
                       for i in range(ND)]
                for d in range(ND):
                    nc.sync.dma_start(kts[d][:], kT.ap()[ts(d, P), :])
                    nc.gpsimd.dma_start(wks[d][:], wkT.ap()[ts(d, P), :])
                for kc in range(S // 512):
                    for e in range(NE):
                        ps = psum.tile([P, 512], F32, tag="mm",
                                       name=f"ps_k{e}_{kc}")
                        for d in range(ND):
                            nc.tensor.matmul(ps[:], wks[d][:, ts(e, P)],
                                             kts[d][:, ts(kc, 512)],
                                             start=(d == 0), stop=(d == ND - 1))
                        kt_c = ktb.tile([P, 512], F32R, tag="ktb",
                                        name=f"ktb{e}_{kc}")
                        nc.vector.tensor_copy(kt_c[:], ps[:])
                        nc.sync.dma_start(kt_spill[e][:, ts(kc, 512)], kt_c[:])

            # ---- P3: scores ST[k,q] (KT streamed); V proj after -------
            vp = ctx.enter_context(tc.tile_pool(name="vp", bufs=NK))
            V = [vp.tile([P, D], BF16, tag="v", name=f"v{i}") for i in range(NK)]
            with tc.tile_pool(name="ktsl", bufs=3) as ktsl, \
                 tc.tile_pool(name="stb", bufs=3) as stb, \
                 tc.tile_pool(name="kbin", bufs=ND) as kbin, \
                 tc.tile_pool(name="wvb", bufs=ND) as wvb:
                # prefetch V-projection inputs on the gpsimd DMA queue
                kbs = [kbin.tile([P, S], BF16, tag="kbin", name=f"kbin{i}")
                       for i in range(ND)]
                wvs = [wvb.tile([P, D], BF16, tag="wvb", name=f"wvb{i}")
                       for i in range(ND)]
                for d in range(ND):
                    nc.gpsimd.dma_start(kbs[d][:], kTb.ap()[ts(d, P), :])
                    nc.gpsimd.dma_start(wvs[d][:], wvTb.ap()[ts(d, P), :])

                for k in range(NK):
                    kt_k = ktsl.tile([P, D], F32R, tag="ktsl", name=f"ktsl{k}")
                    for e in range(NE):
                        nc.sync.dma_start(kt_k[:, ts(e, P)],
                                          kt_spill[e][:, ts(k, P)])
                    st_k = stb.tile([P, SQ], F32, tag="stb", name=f"stb{k}")
                    for qh in range(SQ // 512):
                        ps = psum.tile([P, 512], F32, tag="mm",
                                       name=f"ps_s{k}_{qh}")
                        for e in range(NE):
                            nc.tensor.matmul(ps[:], kt_k[:, ts(e, P)],
                                             QTr[e][:, ts(qh, 512)],
                                             start=(e == 0), stop=(e == NE - 1))
                        nc.vector.tensor_copy(st_k[:, ts(qh, 512)], ps[:])
                    if k == 0:
                        nc.vector.tensor_copy(macc[:], st_k[:])
                    else:
                        nc.vector.tensor_max(macc[:], macc[:], st_k[:])
                    nc.sync.dma_start(st_spill[k][:], st_k[:])

                # V projection: no softmax deps -> fills PE during reduce
                for k in range(NK):
                    for eh in range(D // 512):
                        ps = psum.tile([P, 512], F32, tag="mm",
                                       name=f"ps_v{k}_{eh}")
                        for d in range(ND):
                            nc.tensor.matmul(ps[:], kbs[d][:, ts(k, P)],
                                             wvs[d][:, ts(eh, 512)],
                                             start=(d == 0), stop=(d == ND - 1))
                        nc.vector.tensor_copy(V[k][:, ts(eh, 512)], ps[:])

            # ---- P3b: partition halving max -> broadcast row max ------
            tmp = maxp.tile([64, SQ], F32)
            w = 64
            while w >= 1:
                nc.sync.dma_start(tmp[0:w, :], macc[w:2 * w, :])
                nc.vector.tensor_max(macc[0:w, :], macc[0:w, :], tmp[0:w, :])
                w //= 2
            m_row_r = maxp.tile([1, SQ], F32R)
            nc.vector.tensor_copy(m_row_r[:], macc[0:1, :])
            m_bc = maxp.tile([P, SQ], F32)
            for qh in range(SQ // 512):
                ps = psum.tile([P, 512], F32, tag="mm", name=f"ps_m{qh}")
                nc.tensor.matmul(ps[:], ones_r[:], m_row_r[:, ts(qh, 512)],
                                 start=True, stop=True)
                nc.vector.tensor_copy(m_bc[:, ts(qh, 512)], ps[:])

            # ---- P4a: E = exp(ST - m) in bf16 -------------------------
            ep = ctx.enter_context(tc.tile_pool(name="ep", bufs=NK))
            E = [ep.tile([P, SQ], BF16, tag="e", name=f"e{i}") for i in range(NK)]
            with tc.tile_pool(name="stin", bufs=3) as stin, \
                 tc.tile_pool(name="subp", bufs=3) as subp:
                for k in range(NK):
                    st_k = stin.tile([P, SQ], F32, tag="stin", name=f"stin{k}")
                    nc.sync.dma_start(st_k[:], st_spill[k][:])
                    sub = subp.tile([P, SQ], F32, tag="sub", name=f"sub{k}")
                    nc.vector.tensor_sub(sub[:], st_k[:], m_bc[:])
                    nc.scalar.activation(E[k][:], sub[:],
                                         mybir.ActivationFunctionType.Exp)

            # ---- P4c: l[q] row sums + reciprocal ----------------------
            lp = ctx.enter_context(tc.tile_pool(name="lp", bufs=NQC))
            recip_l = [lp.tile([P, 1], F32, tag="recip", name=f"recip{i}")
                       for i in range(NQC)]
            for qc in range(NQC):
                pl = psl.tile([P, 1], F32, tag="lsum", name=f"pl{qc}")
                for k in range(NK):
                    nc.tensor.matmul(pl[:], E[k][:, ts(qc, P)], ones_b[:],
                                     start=(k == 0), stop=(k == NK - 1))
                nc.vector.reciprocal(recip_l[qc][:], pl[:])

            # ---- P4d: O = (E.T @ V) * recip_l -> DRAM -----------------
            for qc in range(NQC):
                for eh in range(D // 512):
                    ps = psum.tile([P, 512], F32, tag="mm", name=f"ps_o{qc}_{eh}")
                    for k in range(NK):
                        nc.tensor.matmul(ps[:], E[k][:, ts(qc, P)],
                                         V[k][:, ts(eh, 512)],
                                         start=(k == 0), stop=(k == NK - 1))
                    ot = outp.tile([P, 512], F32, tag="ot", name=f"ot{qc}_{eh}")
                    nc.vector.tensor_scalar_mul(ot[:], ps[:], recip_l[qc][:])
                    nc.sync.dma_start(out.ap()[ts(qc, P), ts(eh, 512)], ot[:])

    nc.compile()
    _NC_CACHE["nc"] = nc
    return nc


def kernel(query, key, Wq, Wk, Wv):
    query = np.asarray(query, dtype=np.float32)
    key = np.asarray(key, dtype=np.float32)
    wqT = np.ascontiguousarray(np.asarray(Wq, dtype=np.float32).T)
    wkT = np.ascontiguousarray(np.asarray(Wk, dtype=np.float32).T)
    wvTb = np.ascontiguousarray(np.asarray(Wv, dtype=np.float32).T).astype(
        ml_dtypes.bfloat16)

    in_maps = []
    for c in range(N_CORES):
        b, h = c // 2, c % 2
        qTn = np.ascontiguousarray(query[b, h * SQ:(h + 1) * SQ, :].T)
        kTn = np.ascontiguousarray(key[b].T)
        in_maps.append({
            "qT": qTn, "kT": kTn, "wqT": wqT, "wkT": wkT,
            "kTb": kTn.astype(ml_dtypes.bfloat16), "wvTb": wvTb,
        })

    nc = _build()
    res = run_bass_kernel_spmd(nc, in_maps, core_ids=list(range(N_CORES)))
    outv = np.empty((B, S, D), dtype=np.float32)
    for c in range(N_CORES):
        b, h = c // 2, c % 2
        outv[b, h * SQ:(h + 1) * SQ, :] = res.results[c]["out"]
    return outv
